# revision 2
# baseline (speedup 1.0000x reference)
"""Trainium2 Bass kernel for nn_AsyncTFBase (dense_mlp, 15-head low-rank
bilinear transformer head bank).

Strategy:
- Data-parallel over the batch axis B=64: each of the 8 NeuronCores gets 8
  batch columns (all T=32 timesteps), i.e. 256 tokens.
- All compute in fp16 on the TensorEngine (fp32 PSUM accumulation); weights
  are converted host-side to fp16 and pre-tiled so every device DMA is a
  contiguous per-partition read.
- Feature-major layout ([feature, token]) for all matmuls; the per-token
  rank-R bilinears run on the VectorEngine with broadcast access patterns
  after a PE-transpose to token-major.
- The "temporal" heads (a[t-1] paired with b[t]) are handled host-side by
  passing a second, time-shifted copy of the input features: the a-side
  module of each temporal pair simply consumes the shifted input, making
  every bilinear a same-time bilinear on device.
"""
import numpy as np

import concourse.bass as bass
import concourse.mybir as mybir
import concourse.tile as tile
from concourse import bacc
from concourse.masks import make_identity

# ---------------- problem constants (hardcoded) ----------------
D, T, B = 2048, 32, 64
S, O, V = 16, 38, 33
R, H = 5, 1000
NCORES = 8
BL = B // NCORES          # batch columns per core
NTOK = T * BL             # tokens per core (t-major: tok = t*BL + b)
KC = D // 128             # 16 contraction chunks over D
HCH = [(c * 128, min(128, H - c * 128)) for c in range((H + 127) // 128)]
O2 = 192                  # padded l2 out width
OCH = [(0, 128), (128, 64)]
FP16 = mybir.dt.float16
FP32 = mybir.dt.float32

# 24 head modules in reference order; (name, out_dim)
_HEADS = [
    ("so_a", S * R), ("so_b", R * O), ("ov_a", O * R), ("ov_b", R * V),
    ("vs_a", V * R), ("vs_b", R * S), ("ss_a", S * R), ("ss_b", R * S),
    ("oo_a", O * R), ("oo_b", R * O), ("vv_a", V * R), ("vv_b", R * V),
    ("so_t_a", S * R), ("so_t_b", R * O), ("ov_t_a", O * R), ("ov_t_b", R * V),
    ("vs_t_a", V * R), ("vs_t_b", R * S), ("os_t_a", O * R), ("os_t_b", R * S),
    ("vo_t_a", V * R), ("vo_t_b", R * O), ("sv_t_a", S * R), ("sv_t_b", R * V),
]
# (a_idx, b_idx, ca, cb, temporal)
PAIRS = [
    (0, 1, S, O, False), (2, 3, O, V, False), (4, 5, V, S, False),
    (6, 7, S, S, True), (8, 9, O, O, True), (10, 11, V, V, True),
    (12, 13, S, O, True), (14, 15, O, V, True), (16, 17, V, S, True),
    (18, 19, O, S, True), (20, 21, V, O, True), (22, 23, S, V, True),
]
PAIR_NAMES = ["so", "ov", "vs", "ss", "oo", "vv",
              "so_t", "ov_t", "vs_t", "os_t", "vo_t", "sv_t"]


def _feat_chunks(n):
    """Split n feature rows into the (oc_chunk, rows, col_offset) pieces they
    occupy in the [128, 2, NTOK] padded-l2 layout."""
    out = []
    off = 0
    for oc, (ostart, orows) in enumerate(OCH):
        r = min(n - off, orows)
        if r <= 0:
            break
        out.append((oc, r, off))
        off += r
    return out


def build_nc(repeat=1):
    nc = bacc.Bacc(None, target_bir_lowering=False)

    fT = nc.declare_dram_parameter("fT", [128, KC, NTOK], FP16, isOutput=False)
    fTs = nc.declare_dram_parameter("fTs", [128, KC, NTOK], FP16, isOutput=False)
    W1 = nc.declare_dram_parameter("W1", [24, 128, KC * H], FP16, isOutput=False)
    B1 = nc.declare_dram_parameter("B1", [24, 128, 8], FP32, isOutput=False)
    W2 = nc.declare_dram_parameter("W2", [24, 128, 8 * O2], FP16, isOutput=False)
    B2 = nc.declare_dram_parameter("B2", [24, 128, 2], FP32, isOutput=False)
    s1W = nc.declare_dram_parameter("s1W", [128, KC * H], FP16, isOutput=False)
    s1b = nc.declare_dram_parameter("s1b", [128, 8], FP32, isOutput=False)
    s2W = nc.declare_dram_parameter("s2W", [128, 8 * H], FP16, isOutput=False)
    s2b = nc.declare_dram_parameter("s2b", [128, 8], FP32, isOutput=False)
    s3W = nc.declare_dram_parameter("s3W", [128, 8 * S], FP16, isOutput=False)
    s3b = nc.declare_dram_parameter("s3b", [128, 1], FP32, isOutput=False)
    ovW = nc.declare_dram_parameter("ovW", [128, KC * (O + V)], FP16,
                                    isOutput=False)
    ovb = nc.declare_dram_parameter("ovb", [128, 1], FP32, isOutput=False)

    youts = []
    for pname, (_, _, ca, cb, _) in zip(PAIR_NAMES, PAIRS):
        youts.append(nc.declare_dram_parameter(
            f"y_{pname}", [NTOK, ca * cb], FP32, isOutput=True))
    s_out = nc.declare_dram_parameter("s_out", [S, NTOK], FP32, isOutput=True)
    ov_out = nc.declare_dram_parameter("ov_out", [O + V, NTOK], FP32,
                                       isOutput=True)

    gmax = max(ca * cb * R for (_, _, ca, cb, _) in PAIRS)
    ymax = max(ca * cb for (_, _, ca, cb, _) in PAIRS)

    with tile.TileContext(nc) as tc:
        with (
            tc.tile_pool(name="const", bufs=1) as cpool,
            tc.tile_pool(name="w1p", bufs=2) as w1p,
            tc.tile_pool(name="w2p", bufs=2) as w2p,
            tc.tile_pool(name="bp", bufs=4) as bp,
            tc.tile_pool(name="hp", bufs=2) as hp,
            tc.tile_pool(name="zp", bufs=3) as zp,
            tc.tile_pool(name="tokp", bufs=6) as tokp,
            tc.tile_pool(name="gp", bufs=2) as gpool,
            tc.tile_pool(name="yp", bufs=2) as ypool,
            tc.tile_pool(name="mm", bufs=4, space="PSUM") as mmp,
            tc.tile_pool(name="tp", bufs=2, space="PSUM") as tpp,
        ):
            ident = cpool.tile([128, 128], FP16)
            make_identity(nc, ident[:])

            for _rep in range(repeat):
                fT_sb = cpool.tile([128, KC, NTOK], FP16, name="fT_sb")
                fTs_sb = cpool.tile([128, KC, NTOK], FP16, name="fTs_sb")
                nc.sync.dma_start(out=fT_sb[:], in_=fT[:])
                nc.sync.dma_start(out=fTs_sb[:], in_=fTs[:])

                def l1(rhs_sb, w1_src, b1_src):
                    """relu(x @ W1 + b1), feature-major h [128, 8, NTOK]."""
                    w1t = w1p.tile([128, KC, H], FP16, tag="w1")
                    nc.sync.dma_start(
                        out=w1t[:],
                        in_=w1_src.rearrange("p (a b) -> p a b", b=H))
                    b1t = bp.tile([128, 8], FP32, tag="b1")
                    nc.sync.dma_start(out=b1t[:], in_=b1_src)
                    h_t = hp.tile([128, 8, NTOK], FP16, tag="h")
                    for c, (cs, rows) in enumerate(HCH):
                        ps = mmp.tile([128, NTOK], FP32, tag="mm")
                        for k in range(KC):
                            nc.tensor.matmul(
                                ps[:rows, :], w1t[:, k, cs:cs + rows],
                                rhs_sb[:, k, :],
                                start=(k == 0), stop=(k == KC - 1))
                        nc.scalar.activation(
                            h_t[:rows, c, :], ps[:rows, :],
                            mybir.ActivationFunctionType.Relu,
                            bias=b1t[:rows, c:c + 1])
                    return h_t

                def l2(h_t, w2_src, b2_src):
                    """h @ W2 + b2, feature-major z [128, 2, NTOK] fp16."""
                    w2t = w2p.tile([128, 8, O2], FP16, tag="w2")
                    nc.sync.dma_start(
                        out=w2t[:],
                        in_=w2_src.rearrange("p (a b) -> p a b", b=O2))
                    b2t = bp.tile([128, 2], FP32, tag="b2")
                    nc.sync.dma_start(out=b2t[:], in_=b2_src)
                    zt = zp.tile([128, 2, NTOK], FP16, tag="z")
                    for oc, (ostart, orows) in enumerate(OCH):
                        ps = mmp.tile([128, NTOK], FP32, tag="mm")
                        for c, (cs, rows) in enumerate(HCH):
                            nc.tensor.matmul(
                                ps[:orows, :],
                                w2t[:rows, c, ostart:ostart + orows],
                                h_t[:rows, c, :],
                                start=(c == 0), stop=(c == len(HCH) - 1))
                        nc.vector.tensor_scalar_add(
                            zt[:orows, oc, :], ps[:orows, :],
                            b2t[:orows, oc:oc + 1])
                    return zt

                def to_tok_major(zt, nfeat, tt):
                    """Transpose z^T [nfeat, tt-th 128 tokens] -> [128, nfeat]."""
                    tok = tokp.tile([128, O2], FP16, tag="tok")
                    for (oc, r0, co) in _feat_chunks(nfeat):
                        tp = tpp.tile([128, 128], FP16, tag="tp")
                        nc.tensor.transpose(
                            tp[:, :r0], zt[:r0, oc, tt * 128:(tt + 1) * 128],
                            ident[:r0, :r0])
                        nc.vector.tensor_copy(tok[:, co:co + r0], tp[:, :r0])
                    return tok

                def run_module(m, rhs_sb):
                    return l2(l1(rhs_sb, W1[m], B1[m]), W2[m], B2[m])

                for (ai, bi, ca, cb, temporal), ydram in zip(PAIRS, youts):
                    za = run_module(ai, fTs_sb if temporal else fT_sb)
                    zb = run_module(bi, fT_sb)
                    for tt in range(NTOK // 128):
                        atok = to_tok_major(za, ca * R, tt)
                        btok = to_tok_major(zb, R * cb, tt)
                        a0 = atok[:]
                        b0 = btok[:]
                        # G[p,(i,j,r)] = a[p,i*R+r] * b[p,r*cb+j]
                        a_ap = bass.AP(a0.tensor, a0.offset,
                                       [a0.ap[0], [R, ca], [0, cb], [1, R]])
                        b_ap = bass.AP(b0.tensor, b0.offset,
                                       [b0.ap[0], [0, ca], [1, cb], [cb, R]])
                        g = gpool.tile([128, ca * cb * R], FP16, tag="g",
                                       padded_shape=[128, gmax])
                        g0 = g[:]
                        g_ap = bass.AP(g0.tensor, g0.offset,
                                       [g0.ap[0], [cb * R, ca], [R, cb], [1, R]])
                        nc.vector.tensor_tensor(g_ap, a_ap, b_ap,
                                                mybir.AluOpType.mult)
                        y = ypool.tile([128, ca * cb], FP32, tag="y",
                                       padded_shape=[128, ymax])
                        nc.vector.tensor_reduce(
                            y[:], g[:].rearrange("p (ij r) -> p ij r", r=R),
                            axis=mybir.AxisListType.X, op=mybir.AluOpType.add)
                        nc.sync.dma_start(
                            out=ydram[tt * 128:(tt + 1) * 128, :], in_=y[:])

                # ---- s path: 3-layer MLP ----
                h1 = l1(fT_sb, s1W[:], s1b[:])
                s2t = w1p.tile([128, 8, H], FP16, tag="w1")
                nc.sync.dma_start(
                    out=s2t[:], in_=s2W[:].rearrange("p (a b) -> p a b", b=H))
                s2bt = bp.tile([128, 8], FP32, tag="b1")
                nc.sync.dma_start(out=s2bt[:], in_=s2b[:])
                h2 = hp.tile([128, 8, NTOK], FP16, tag="h")
                for oc, (cs2, orows) in enumerate(HCH):
                    ps = mmp.tile([128, NTOK], FP32, tag="mm")
                    for c, (cs, rows) in enumerate(HCH):
                        nc.tensor.matmul(
                            ps[:orows, :], s2t[:rows, c, cs2:cs2 + orows],
                            h1[:rows, c, :],
                            start=(c == 0), stop=(c == len(HCH) - 1))
                    nc.scalar.activation(
                        h2[:orows, oc, :], ps[:orows, :],
                        mybir.ActivationFunctionType.Relu,
                        bias=s2bt[:orows, oc:oc + 1])
                s3t = cpool.tile([128, 8, S], FP16, name="s3t")
                nc.sync.dma_start(
                    out=s3t[:], in_=s3W[:].rearrange("p (a b) -> p a b", b=S))
                s3bt = cpool.tile([128, 1], FP32, name="s3bt")
                nc.sync.dma_start(out=s3bt[:], in_=s3b[:])
                ps = mmp.tile([128, NTOK], FP32, tag="mm")
                for c, (cs, rows) in enumerate(HCH):
                    nc.tensor.matmul(ps[:S, :], s3t[:rows, c, :],
                                     h2[:rows, c, :],
                                     start=(c == 0), stop=(c == len(HCH) - 1))
                s_sb = tokp.tile([128, NTOK], FP32, tag="sout", bufs=1)
                nc.vector.tensor_scalar_add(s_sb[:S, :], ps[:S, :],
                                            s3bt[:S, :])
                nc.sync.dma_start(out=s_out[:], in_=s_sb[:S, :])

                # ---- o and v linear heads (fused) ----
                ovt = cpool.tile([128, KC, O + V], FP16, name="ovt")
                nc.sync.dma_start(
                    out=ovt[:], in_=ovW[:].rearrange("p (a b) -> p a b",
                                                     b=O + V))
                ovbt = cpool.tile([128, 1], FP32, name="ovbt")
                nc.sync.dma_start(out=ovbt[:], in_=ovb[:])
                ps = mmp.tile([128, NTOK], FP32, tag="mm")
                for k in range(KC):
                    nc.tensor.matmul(ps[:O + V, :], ovt[:, k, :],
                                     fT_sb[:, k, :],
                                     start=(k == 0), stop=(k == KC - 1))
                ov_sb = tokp.tile([128, NTOK], FP32, tag="sout", bufs=1)
                nc.vector.tensor_scalar_add(ov_sb[:O + V, :], ps[:O + V, :],
                                            ovbt[:O + V, :])
                nc.sync.dma_start(out=ov_out[:], in_=ov_sb[:O + V, :])

    nc.compile()
    return nc


# ---------------- host-side packing ----------------

def _chunk_pad_rows(w, nrows_pad):
    """[nrows, cols] -> [128, ceil/128, cols] with zero pad, as
    [p, chunk, col] then flattened to [128, chunk*cols]."""
    nr, ncol = w.shape
    nch = nrows_pad // 128
    wp = np.zeros((nch * 128, ncol), w.dtype)
    wp[:nr] = w
    return np.ascontiguousarray(
        wp.reshape(nch, 128, ncol).transpose(1, 0, 2).reshape(128, nch * ncol))


def _pack_bias(b, nch):
    out = np.zeros((128, nch), np.float32)
    n = b.shape[0]
    for c in range(nch):
        r = min(128, max(0, n - c * 128))
        if r:
            out[:r, c] = b[c * 128:c * 128 + r]
    return out


def _prep_inputs(rgb_feat, params):
    f = np.asarray(rgb_feat, np.float32)           # [T, B, D]
    fs = np.concatenate([f[:1], f[:-1]], axis=0)   # time-shifted
    w1_l, b1_l, w2_l, b2_l = [], [], [], []
    for name, outdim in _HEADS:
        p = params[name]
        w1 = np.asarray(p["l1"]["W"], np.float32).astype(np.float16)
        w1_l.append(_chunk_pad_rows(w1, D))
        b1_l.append(_pack_bias(np.asarray(p["l1"]["b"], np.float32), 8))
        w2 = np.asarray(p["l2"]["W"], np.float32)
        w2p = np.zeros((H, O2), np.float32)
        w2p[:, :outdim] = w2
        w2_l.append(_chunk_pad_rows(w2p.astype(np.float16), 1024))
        b2p = np.zeros(O2, np.float32)
        b2p[:outdim] = np.asarray(p["l2"]["b"], np.float32)
        b2_l.append(_pack_bias(b2p, 2))
    common = {
        "W1": np.ascontiguousarray(np.stack(w1_l)),
        "B1": np.ascontiguousarray(np.stack(b1_l)),
        "W2": np.ascontiguousarray(np.stack(w2_l)),
        "B2": np.ascontiguousarray(np.stack(b2_l)),
        "s1W": _chunk_pad_rows(
            np.asarray(params["s1"]["W"], np.float32).astype(np.float16), D),
        "s1b": _pack_bias(np.asarray(params["s1"]["b"], np.float32), 8),
        "s2W": _chunk_pad_rows(
            np.asarray(params["s2"]["W"], np.float32).astype(np.float16), 1024),
        "s2b": _pack_bias(np.asarray(params["s2"]["b"], np.float32), 8),
        "s3W": _chunk_pad_rows(
            np.asarray(params["s3"]["W"], np.float32).astype(np.float16), 1024),
        "s3b": _pack_bias(np.asarray(params["s3"]["b"], np.float32), 1),
        "ovW": _chunk_pad_rows(
            np.concatenate([np.asarray(params["o"]["W"], np.float32),
                            np.asarray(params["v"]["W"], np.float32)],
                           axis=1).astype(np.float16), D),
        "ovb": _pack_bias(
            np.concatenate([np.asarray(params["o"]["b"], np.float32),
                            np.asarray(params["v"]["b"], np.float32)]), 1),
    }

    in_maps = []
    for c in range(NCORES):
        fc = f[:, c * BL:(c + 1) * BL, :].reshape(NTOK, D)
        fsc = fs[:, c * BL:(c + 1) * BL, :].reshape(NTOK, D)

        def tok_major_T(x):   # [NTOK, D] -> [128, KC, NTOK] fp16
            xT = x.T.astype(np.float16)  # [D, NTOK]
            return np.ascontiguousarray(
                xT.reshape(KC, 128, NTOK).transpose(1, 0, 2))

        m = dict(common)
        m["fT"] = tok_major_T(fc)
        m["fTs"] = tok_major_T(fsc)
        in_maps.append(m)
    return in_maps


_NC_CACHE = {}


def _get_nc(repeat=1):
    if repeat not in _NC_CACHE:
        _NC_CACHE[repeat] = build_nc(repeat)
    return _NC_CACHE[repeat]


def kernel(rgb_feat, params):
    from concourse.bass_utils import run_bass_kernel_spmd

    in_maps = _prep_inputs(rgb_feat, params)
    nc = _get_nc(1)
    res = run_bass_kernel_spmd(nc, in_maps, list(range(NCORES)), trace=False)

    per_core = res.results
    s = np.concatenate(
        [per_core[c]["s_out"].T.reshape(T, BL, S) for c in range(NCORES)],
        axis=1)
    ov = [per_core[c]["ov_out"] for c in range(NCORES)]
    o = np.concatenate([x[:O].T.reshape(T, BL, O) for x in ov], axis=1)
    v = np.concatenate([x[O:].T.reshape(T, BL, V) for x in ov], axis=1)
    outs = [s, o, v]
    for pname, (_, _, ca, cb, _) in zip(PAIR_NAMES, PAIRS):
        outs.append(np.concatenate(
            [per_core[c][f"y_{pname}"].reshape(T, BL, ca, cb)
             for c in range(NCORES)], axis=1))
    return tuple(np.ascontiguousarray(x, dtype=np.float32) for x in outs)
